# revision 15
# baseline (speedup 1.0000x reference)
"""Trainium2 Bass kernel for nn_CNNStateEncoder (dense_cnn).

Network per row (B*S rows, 8 features each):
  conv1 2x2 on [1,2,4] -> 32ch x [1,3]   == h1[96]  = A1[96,8]  @ x[8],  relu(+b1)
  conv2 1x2 on [32,1,3] -> 32ch x [1,2]  == h2[64]  = A2[64,96] @ h1,    relu(+b2)
  linear 64->64                          == out[64] = Wp[64,64] @ h2 + bp

Data parallel over 8 cores, 65536 rows/core, 2048-row tiles. Feature-major
chain: rows live in the matmul free dim. The host pre-transposes and
4x-replicates x into the xt layout (xt[32q+8g+f, 32a+v] = x[512q+16v+a, f]
per tile), so the device does no input shuffling at all. Per tile:
  - conv1: 4 packed K=8 matmuls, contiguous rhs streams (row permutation
    undone host-side)
  - relu1 (+b1): single Scalar ACT op [96, 2048]
  - conv2: K=96 matmuls, 2 col groups concurrent, one [128,1024] psum
  - relu2 (+b2): single DVE tensor_scalar [128, 1024]
  - linear: h2 chunks as stationary -> row-major psum [128,1024]
  - bias+cast: single DVE tensor_tensor -> bf16, contiguous 2KB/partition
    store; host unpermutes rows and casts to f32
"""

import numpy as np
import ml_dtypes

B, S, FEAT, OUT = 64, 8192, 8, 64
NCORES = 8
ROWS_TOTAL = B * S
ROWS_CORE = ROWS_TOTAL // NCORES  # 65536
TILE_ROWS = 2048
NTILES = ROWS_CORE // TILE_ROWS  # 32

BF16 = ml_dtypes.bfloat16

# ---------------------------------------------------------------------------
# host-side permutations
#
# conv1 streams xt columns in natural order, so psum position s holds row
# rho(s) = 512*(s>>9) + 16*(s&31) + ((s&511)>>5)  (within its 2048-row tile).
# The store writes psum (p, chunk r) -> dram slot 16p+r, i.e. dram slot
# d = 16p+r holds true row rho(128r+p).  INV below maps dram order back.
# ---------------------------------------------------------------------------

def _inv_perm():
    d = np.arange(TILE_ROWS)
    p, c = d >> 4, d & 15          # dram slot 16p+c <- psum (p, chunk c)
    h, X, c4 = c >> 3, (c >> 2) & 1, c & 3
    # conv1 quadrant q feeds: (X,h)=(0,0)->q0, (0,1)->q2, (1,0)->q1, (1,1)->q3
    q = np.choose(2 * X + h, [0, 2, 1, 3])
    i = 128 * c4 + p               # position within the quadrant's 512 cols
    rho = 512 * q + 16 * (i & 31) + (i >> 5)
    inv = np.empty(TILE_ROWS, np.int64)
    inv[rho] = d
    return inv

INV = _inv_perm()


def make_xt(x_core):
    """[ROWS_CORE, 8] bf16 -> [NTILES*128, 512] bf16 in device xt layout."""
    a = x_core.reshape(NTILES, 4, 32, 16, 8)        # (t, q, v, a, f)
    a = a.transpose(0, 1, 4, 3, 2)                  # (t, q, f, a, v)
    a = np.broadcast_to(a[:, :, None], (NTILES, 4, 4, 8, 16, 32))  # (t,q,g,f,a,v)
    return np.ascontiguousarray(a).reshape(NTILES * 128, 512)


# ---------------------------------------------------------------------------
# numpy-side weight packing
# ---------------------------------------------------------------------------

def pack_weights(W1, b1, W2, b2, Wp, bp):
    W1 = np.asarray(W1, np.float32)
    W2 = np.asarray(W2, np.float32)
    Wp = np.asarray(Wp, np.float32)
    b1 = np.asarray(b1, np.float32)
    b2 = np.asarray(b2, np.float32)
    bp = np.asarray(bp, np.float32)

    # A1 [96, 8]: h1[o*3+j] = sum_{kh,kw} x[kh*4 + j + kw] * W1[o,0,kh,kw]
    A1 = np.zeros((96, 8), np.float32)
    for o in range(32):
        for j in range(3):
            for kh in range(2):
                for kw in range(2):
                    A1[o * 3 + j, kh * 4 + j + kw] += W1[o, 0, kh, kw]
    b1_96 = np.repeat(b1, 3).astype(np.float32)

    # A2 [64, 96]: h2[c*2+w] = sum_{i,kw} h1[i*3 + w + kw] * W2[c,i,0,kw]
    A2 = np.zeros((64, 96), np.float32)
    for c in range(32):
        for w in range(2):
            for i in range(32):
                for kw in range(2):
                    A2[c * 2 + w, i * 3 + w + kw] += W2[c, i, 0, kw]
    b2_64 = np.repeat(b2, 2).astype(np.float32)

    a1t = np.zeros((128, 96), np.float32)
    for q in range(4):
        a1t[32 * q:32 * q + 8, :] = A1.T
    a2t = np.zeros((96, 128), np.float32)
    a2t[:, 0:64] = A2.T
    a2t[:, 64:128] = A2.T
    wpt = np.zeros((128, 64), np.float32)
    wpt[0:64, :] = Wp.T
    wpt[64:128, :] = Wp.T
    b1c = b1_96.reshape(96, 1)
    b2c = np.concatenate([b2_64, b2_64]).reshape(128, 1)
    bpb = np.tile(bp, (128, TILE_ROWS // 128))  # [128, 1024]

    return {
        "a1t": a1t.astype(BF16),
        "a2t": a2t.astype(BF16),
        "wpt": wpt.astype(BF16),
        "b1c": b1c,
        "b2c": b2c,
        "bpb": bpb.astype(np.float32),
    }


# ---------------------------------------------------------------------------
# bass module
# ---------------------------------------------------------------------------

def build_nc(rows=ROWS_CORE):
    import concourse.bass as bass
    import concourse.bacc as bacc
    import concourse.mybir as mybir
    import concourse.tile as tile

    f32 = mybir.dt.float32
    bf16 = mybir.dt.bfloat16
    Relu = mybir.ActivationFunctionType.Relu
    Alu = mybir.AluOpType

    assert rows % TILE_ROWS == 0
    ntiles = rows // TILE_ROWS

    nc = bacc.Bacc(None, target_bir_lowering=False)

    xt_d = nc.dram_tensor("xt", [ntiles * 128, 512], bf16, kind="ExternalInput")
    a1t_d = nc.dram_tensor("a1t", [128, 96], bf16, kind="ExternalInput")
    a2t_d = nc.dram_tensor("a2t", [96, 128], bf16, kind="ExternalInput")
    wpt_d = nc.dram_tensor("wpt", [128, 64], bf16, kind="ExternalInput")
    b1c_d = nc.dram_tensor("b1c", [96, 1], f32, kind="ExternalInput")
    b2c_d = nc.dram_tensor("b2c", [128, 1], f32, kind="ExternalInput")
    bpb_d = nc.dram_tensor("bpb", [128, 1024], f32, kind="ExternalInput")
    out_d = nc.dram_tensor("out", [rows, OUT], bf16, kind="ExternalOutput")

    with tile.TileContext(nc) as tc:
        with (
            tc.tile_pool(name="consts", bufs=1) as cpool,
            tc.tile_pool(name="xt", bufs=4) as xtpool,
            tc.tile_pool(name="h1s", bufs=3) as h1pool,
            tc.tile_pool(name="h2s", bufs=3) as h2pool,
            tc.tile_pool(name="osb", bufs=3) as opool,
            tc.tile_pool(name="ps_h1", bufs=1, space="PSUM") as ps_h1,
            tc.tile_pool(name="ps_bk", bufs=2, space="PSUM") as ps_bk,
        ):
            a1t = cpool.tile([128, 96], bf16)
            a2t = cpool.tile([96, 128], bf16)
            wpt = cpool.tile([128, 64], bf16)
            b1c = cpool.tile([96, 1], f32)
            b2c = cpool.tile([128, 1], f32)
            bpb = cpool.tile([128, 1024], f32)
            nc.sync.dma_start(a1t[:], a1t_d[:])
            nc.sync.dma_start(a2t[:], a2t_d[:])
            nc.sync.dma_start(wpt[:], wpt_d[:])
            nc.sync.dma_start(b1c[:], b1c_d[:])
            nc.sync.dma_start(b2c[:], b2c_d[:])
            nc.sync.dma_start(bpb[:], bpb_d[:])

            def front(t):
                """load + conv1 + relu1 for tile t; returns h1s."""
                xt = xtpool.tile([128, 512], bf16)
                nc.sync.dma_start(xt[:], xt_d[t * 128:(t + 1) * 128, :])
                # Two independent 2-bank psum tiles: A <- quadrants {0,2},
                # B <- {1,3}.  Each conv2 pair then reads exactly one tile
                # and the two relu1 drains share no tile (no cross-engine
                # serialization).
                h1ps_a = ps_h1.tile([96, 1024], f32)
                h1ps_b = ps_h1.tile([96, 1024], f32)
                for q, (ps, col) in enumerate(
                    ((h1ps_a, 0), (h1ps_b, 0), (h1ps_a, 512), (h1ps_b, 512))
                ):
                    nc.tensor.matmul(
                        ps[:, col:col + 512],
                        a1t[32 * q:32 * q + 8, :],
                        xt[32 * q:32 * q + 8, :],
                        tile_position=(32 * q, 0),
                    )
                h1s_a = h1pool.tile([96, 1024], bf16)
                h1s_b = h1pool.tile([96, 1024], bf16)
                nc.vector.tensor_scalar(
                    h1s_a[:], h1ps_a[:], b1c[:], 0.0, Alu.add, Alu.max
                )
                nc.vector.tensor_scalar(
                    h1s_b[:], h1ps_b[:], b1c[:], 0.0, Alu.add, Alu.max
                )
                return h1s_a, h1s_b

            def mid(t, h1s):
                """conv2 + relu2 for tile t; returns (h2s, outps-tile)."""
                h1s_a, h1s_b = h1s
                # One double-buffered psum tile serves both conv2's output
                # (h2ps) and linear's output (outps) — h2ps is dead once
                # relu2 drains it, so linear reuses the same banks; the
                # second buffer lets conv2(t+1) overlap relu2(t)/linear(t).
                h2ps = ps_bk.tile([128, 1024], f32)
                for lo, h1sx in ((0, h1s_a), (512, h1s_b)):
                    for h in (0, 1):
                        nc.tensor.matmul(
                            h2ps[64 * h:64 * h + 64, lo:lo + 512],
                            a2t[:, 64 * h:64 * h + 64],
                            h1sx[:, 512 * h:512 * h + 512],
                            tile_position=(0, 64 * h),
                        )
                # relu2 (+b2): single Scalar ACT op
                h2s = h2pool.tile([128, 1024], bf16)
                nc.scalar.activation(h2s[:], h2ps[:], Relu, bias=b2c[:])
                return h2s, h2ps

            def tail(t, h2s, outps):
                """linear + out-copy + store for tile t (bias added on host)."""
                n0 = t * TILE_ROWS
                for cc in range(8):
                    for h in (0, 1):
                        c = 8 * h + cc
                        X = (c // 4) % 2
                        col = 512 * X + 128 * (c % 4)
                        nc.tensor.matmul(
                            outps[:, 64 * c:64 * c + 64],
                            h2s[64 * h:64 * h + 64, col:col + 128],
                            wpt[64 * h:64 * h + 64, :],
                            start=(cc == 0),
                            stop=(cc == 7),
                            tile_position=(64 * h, 0),
                        )
                out_sb = opool.tile([128, 1024], bf16)
                nc.scalar.copy(out_sb[:], outps[:])
                nc.sync.dma_start(
                    out_d[n0:n0 + TILE_ROWS, :].rearrange("(p r) j -> p (r j)", p=128),
                    out_sb[:],
                )

            # software pipeline, 2 deep: per iteration emit
            #   front(t+1) -> conv2/relu2(t) -> linear/copy/store(t-1)
            # so each engine runs a single in-order stream:
            #   T: conv1, conv2, linear   S: relu2, copy   V: relu1a, relu1b
            h1s_cur = front(0)
            m_prev = None
            for t in range(ntiles):
                h1s_next = front(t + 1) if t + 1 < ntiles else None
                m_cur = mid(t, h1s_cur)
                if m_prev is not None:
                    tail(t - 1, *m_prev)
                h1s_cur, m_prev = h1s_next, m_cur
            tail(ntiles - 1, *m_prev)

    nc.compile()
    return nc


# ---------------------------------------------------------------------------
# entry point
# ---------------------------------------------------------------------------

_CACHE = {}


def _get_nc(rows=ROWS_CORE):
    if rows not in _CACHE:
        _CACHE[rows] = build_nc(rows)
    return _CACHE[rows]


def make_in_maps(x, W1, b1, W2, b2, Wp, bp):
    x = np.ascontiguousarray(np.asarray(x, np.float32)).reshape(ROWS_TOTAL, FEAT)
    x = x.astype(BF16)
    consts = pack_weights(W1, b1, W2, b2, Wp, bp)
    in_maps = []
    for c in range(NCORES):
        m = dict(consts)
        m["xt"] = make_xt(x[c * ROWS_CORE:(c + 1) * ROWS_CORE])
        in_maps.append(m)
    return in_maps


def postprocess(results, bp):
    out = np.concatenate([np.asarray(r["out"]) for r in results], axis=0)
    out = out.reshape(-1, TILE_ROWS, OUT)[:, INV, :].astype(np.float32)
    out += np.asarray(bp, np.float32)[None, None, :]
    return out.reshape(B, S, OUT)


def kernel(x, W1, b1, W2, b2, Wp, bp):
    from concourse.bass_utils import run_bass_kernel_spmd

    nc = _get_nc()
    in_maps = make_in_maps(x, W1, b1, W2, b2, Wp, bp)
    res = run_bass_kernel_spmd(nc, in_maps, core_ids=list(range(NCORES)))
    return postprocess(res.results, bp)


# revision 19
# speedup vs baseline: 1.0498x; 1.0498x over previous
"""Trainium2 Bass kernel for nn_CNNStateEncoder (dense_cnn).

Network per row (B*S rows, 8 features each):
  conv1 2x2 on [1,2,4] -> 32ch x [1,3]   == h1[96]  = A1[96,8]  @ x[8],  relu(+b1)
  conv2 1x2 on [32,1,3] -> 32ch x [1,2]  == h2[64]  = A2[64,96] @ h1,    relu(+b2)
  linear 64->64                          == out[64] = Wp[64,64] @ h2 + bp

Data parallel over 8 cores, 65536 rows/core, 2048-row tiles. Feature-major
chain: rows live in the matmul free dim. The host pre-transposes and
4x-replicates x into the xt layout (xt[32q+8g+f, 32a+v] = x[512q+16v+a, f]
per tile), so the device does no input shuffling at all. Per tile:
  - conv1: 4 packed K=8 matmuls, contiguous rhs streams (row permutation
    undone host-side)
  - relu1 (+b1): single Scalar ACT op [96, 2048]
  - conv2: K=96 matmuls, 2 col groups concurrent, one [128,1024] psum
  - relu2 (+b2): single DVE tensor_scalar [128, 1024]
  - linear: h2 chunks as stationary -> row-major psum [128,1024]
  - bias+cast: single DVE tensor_tensor -> bf16, contiguous 2KB/partition
    store; host unpermutes rows and casts to f32
"""

import numpy as np
import ml_dtypes

B, S, FEAT, OUT = 64, 8192, 8, 64
NCORES = 8
ROWS_TOTAL = B * S
ROWS_CORE = ROWS_TOTAL // NCORES  # 65536
TILE_ROWS = 2048
NTILES = ROWS_CORE // TILE_ROWS  # 32

BF16 = ml_dtypes.bfloat16

# ---------------------------------------------------------------------------
# host-side permutations
#
# conv1 streams xt columns in natural order, so psum position s holds row
# rho(s) = 512*(s>>9) + 16*(s&31) + ((s&511)>>5)  (within its 2048-row tile).
# The store writes psum (p, chunk r) -> dram slot 16p+r, i.e. dram slot
# d = 16p+r holds true row rho(128r+p).  INV below maps dram order back.
# ---------------------------------------------------------------------------

def _inv_perm():
    d = np.arange(TILE_ROWS)
    p, c = d >> 4, d & 15          # dram slot 16p+c <- psum (p, chunk c)
    h, X, c4 = c >> 3, (c >> 2) & 1, c & 3
    # conv1 quadrant q feeds: (X,h)=(0,0)->q0, (0,1)->q2, (1,0)->q1, (1,1)->q3
    q = np.choose(2 * X + h, [0, 2, 1, 3])
    i = 128 * c4 + p               # position within the quadrant's 512 cols
    rho = 512 * q + 16 * (i & 31) + (i >> 5)
    inv = np.empty(TILE_ROWS, np.int64)
    inv[rho] = d
    return inv

INV = _inv_perm()


def make_xt(x_core):
    """[ROWS_CORE, 8] bf16 -> [NTILES*128, 512] bf16 in device xt layout."""
    a = x_core.reshape(NTILES, 4, 32, 16, 8)        # (t, q, v, a, f)
    a = a.transpose(0, 1, 4, 3, 2)                  # (t, q, f, a, v)
    a = np.broadcast_to(a[:, :, None], (NTILES, 4, 4, 8, 16, 32))  # (t,q,g,f,a,v)
    return np.ascontiguousarray(a).reshape(NTILES * 128, 512)


# ---------------------------------------------------------------------------
# numpy-side weight packing
# ---------------------------------------------------------------------------

def pack_weights(W1, b1, W2, b2, Wp, bp):
    W1 = np.asarray(W1, np.float32)
    W2 = np.asarray(W2, np.float32)
    Wp = np.asarray(Wp, np.float32)
    b1 = np.asarray(b1, np.float32)
    b2 = np.asarray(b2, np.float32)
    bp = np.asarray(bp, np.float32)

    # A1 [96, 8]: h1[o*3+j] = sum_{kh,kw} x[kh*4 + j + kw] * W1[o,0,kh,kw]
    A1 = np.zeros((96, 8), np.float32)
    for o in range(32):
        for j in range(3):
            for kh in range(2):
                for kw in range(2):
                    A1[o * 3 + j, kh * 4 + j + kw] += W1[o, 0, kh, kw]
    b1_96 = np.repeat(b1, 3).astype(np.float32)

    # A2 [64, 96]: h2[c*2+w] = sum_{i,kw} h1[i*3 + w + kw] * W2[c,i,0,kw]
    A2 = np.zeros((64, 96), np.float32)
    for c in range(32):
        for w in range(2):
            for i in range(32):
                for kw in range(2):
                    A2[c * 2 + w, i * 3 + w + kw] += W2[c, i, 0, kw]
    b2_64 = np.repeat(b2, 2).astype(np.float32)

    a1t = np.zeros((128, 96), np.float32)
    for q in range(4):
        a1t[32 * q:32 * q + 8, :] = A1.T
    a2t = np.zeros((96, 128), np.float32)
    a2t[:, 0:64] = A2.T
    a2t[:, 64:128] = A2.T
    wpt = np.zeros((128, 64), np.float32)
    wpt[0:64, :] = Wp.T
    wpt[64:128, :] = Wp.T
    b1c = b1_96.reshape(96, 1)
    b2c = np.concatenate([b2_64, b2_64]).reshape(128, 1)
    bpb = np.tile(bp, (128, TILE_ROWS // 128))  # [128, 1024]

    return {
        "a1t": a1t.astype(BF16),
        "a2t": a2t.astype(BF16),
        "wpt": wpt.astype(BF16),
        "b1c": b1c,
        "b2c": b2c,
        "bpb": bpb.astype(np.float32),
    }


# ---------------------------------------------------------------------------
# bass module
# ---------------------------------------------------------------------------

def build_nc(rows=ROWS_CORE):
    import concourse.bass as bass
    import concourse.bacc as bacc
    import concourse.mybir as mybir
    import concourse.tile as tile

    f32 = mybir.dt.float32
    bf16 = mybir.dt.bfloat16
    Relu = mybir.ActivationFunctionType.Relu
    Alu = mybir.AluOpType

    assert rows % TILE_ROWS == 0
    ntiles = rows // TILE_ROWS

    nc = bacc.Bacc(None, target_bir_lowering=False)

    xt_d = nc.dram_tensor("xt", [ntiles * 128, 512], bf16, kind="ExternalInput")
    a1t_d = nc.dram_tensor("a1t", [128, 96], bf16, kind="ExternalInput")
    a2t_d = nc.dram_tensor("a2t", [96, 128], bf16, kind="ExternalInput")
    wpt_d = nc.dram_tensor("wpt", [128, 64], bf16, kind="ExternalInput")
    b1c_d = nc.dram_tensor("b1c", [96, 1], f32, kind="ExternalInput")
    b2c_d = nc.dram_tensor("b2c", [128, 1], f32, kind="ExternalInput")
    bpb_d = nc.dram_tensor("bpb", [128, 1024], f32, kind="ExternalInput")
    out_d = nc.dram_tensor("out", [rows, OUT], bf16, kind="ExternalOutput")

    with tile.TileContext(nc) as tc:
        with (
            tc.tile_pool(name="consts", bufs=1) as cpool,
            tc.tile_pool(name="xt", bufs=6) as xtpool,
            tc.tile_pool(name="h1s", bufs=4) as h1pool,
            tc.tile_pool(name="h2s", bufs=4) as h2pool,
            tc.tile_pool(name="osb", bufs=4) as opool,
            tc.tile_pool(name="ps_h1", bufs=1, space="PSUM") as ps_h1,
            tc.tile_pool(name="ps_bk", bufs=2, space="PSUM") as ps_bk,
        ):
            a1t = cpool.tile([128, 96], bf16)
            a2t = cpool.tile([96, 128], bf16)
            wpt = cpool.tile([128, 64], bf16)
            b1c = cpool.tile([96, 1], f32)
            b2c = cpool.tile([128, 1], f32)
            bpb = cpool.tile([128, 1024], f32)
            nc.sync.dma_start(a1t[:], a1t_d[:])
            nc.sync.dma_start(a2t[:], a2t_d[:])
            nc.sync.dma_start(wpt[:], wpt_d[:])
            nc.sync.dma_start(b1c[:], b1c_d[:])
            nc.sync.dma_start(b2c[:], b2c_d[:])
            nc.sync.dma_start(bpb[:], bpb_d[:])

            def front(t):
                """load + conv1 + relu1 for tile t; returns h1s."""
                xt = xtpool.tile([128, 512], bf16)
                nc.sync.dma_start(xt[:], xt_d[t * 128:(t + 1) * 128, :])
                # Two independent 2-bank psum tiles: A <- quadrants {0,2},
                # B <- {1,3}.  Each conv2 pair then reads exactly one tile
                # and the two relu1 drains share no tile (no cross-engine
                # serialization).
                h1ps_a = ps_h1.tile([96, 1024], f32)
                h1ps_b = ps_h1.tile([96, 1024], f32)
                for q, (ps, col) in enumerate(
                    ((h1ps_a, 0), (h1ps_b, 0), (h1ps_a, 512), (h1ps_b, 512))
                ):
                    nc.tensor.matmul(
                        ps[:, col:col + 512],
                        a1t[32 * q:32 * q + 8, :],
                        xt[32 * q:32 * q + 8, :],
                        tile_position=(32 * q, 0),
                    )
                h1s_a = h1pool.tile([96, 1024], bf16)
                h1s_b = h1pool.tile([96, 1024], bf16)
                nc.scalar.activation(h1s_a[:], h1ps_a[:], Relu, bias=b1c[:])
                nc.vector.tensor_scalar(
                    h1s_b[:], h1ps_b[:], b1c[:], 0.0, Alu.add, Alu.max
                )
                return h1s_a, h1s_b

            def mid(t, h1s):
                """conv2 + relu2 for tile t; returns (h2s, outps-tile)."""
                h1s_a, h1s_b = h1s
                # One double-buffered psum tile serves both conv2's output
                # (h2ps) and linear's output (outps) — h2ps is dead once
                # relu2 drains it, so linear reuses the same banks; the
                # second buffer lets conv2(t+1) overlap relu2(t)/linear(t).
                h2ps = ps_bk.tile([128, 1024], f32)
                for lo, h1sx in ((0, h1s_a), (512, h1s_b)):
                    for h in (0, 1):
                        nc.tensor.matmul(
                            h2ps[64 * h:64 * h + 64, lo:lo + 512],
                            a2t[:, 64 * h:64 * h + 64],
                            h1sx[:, 512 * h:512 * h + 512],
                            tile_position=(0, 64 * h),
                        )
                # relu2 (+b2): single Scalar ACT op
                h2s = h2pool.tile([128, 1024], bf16)
                nc.scalar.activation(h2s[:], h2ps[:], Relu, bias=b2c[:])
                return h2s, h2ps

            def tail(t, h2s, outps):
                """linear + bias + store for tile t."""
                n0 = t * TILE_ROWS
                for cc in range(8):
                    for h in (0, 1):
                        c = 8 * h + cc
                        X = (c // 4) % 2
                        col = 512 * X + 128 * (c % 4)
                        nc.tensor.matmul(
                            outps[:, 64 * c:64 * c + 64],
                            h2s[64 * h:64 * h + 64, col:col + 128],
                            wpt[64 * h:64 * h + 64, :],
                            start=(cc == 0),
                            stop=(cc == 7),
                            tile_position=(64 * h, 0),
                        )
                out_sb = opool.tile([128, 1024], bf16)
                nc.vector.tensor_tensor(out_sb[:], outps[:], bpb[:], Alu.add)
                nc.sync.dma_start(
                    out_d[n0:n0 + TILE_ROWS, :].rearrange("(p r) j -> p (r j)", p=128),
                    out_sb[:],
                )

            # software-pipelined by one tile: front(t+1) is emitted before
            # the back half of tile t, so FIFO order matches dependencies.
            h1s_cur = front(0)
            for t in range(ntiles):
                h1s_next = front(t + 1) if t + 1 < ntiles else None
                h2s, outps = mid(t, h1s_cur)
                tail(t, h2s, outps)
                h1s_cur = h1s_next

    nc.compile()
    return nc


# ---------------------------------------------------------------------------
# entry point
# ---------------------------------------------------------------------------

_CACHE = {}


def _get_nc(rows=ROWS_CORE):
    if rows not in _CACHE:
        _CACHE[rows] = build_nc(rows)
    return _CACHE[rows]


def make_in_maps(x, W1, b1, W2, b2, Wp, bp):
    x = np.ascontiguousarray(np.asarray(x, np.float32)).reshape(ROWS_TOTAL, FEAT)
    x = x.astype(BF16)
    consts = pack_weights(W1, b1, W2, b2, Wp, bp)
    in_maps = []
    for c in range(NCORES):
        m = dict(consts)
        m["xt"] = make_xt(x[c * ROWS_CORE:(c + 1) * ROWS_CORE])
        in_maps.append(m)
    return in_maps


def postprocess(results, bp=None):
    out = np.concatenate([np.asarray(r["out"]) for r in results], axis=0)
    out = out.reshape(-1, TILE_ROWS, OUT)[:, INV, :]
    return out.reshape(B, S, OUT).astype(np.float32)


def kernel(x, W1, b1, W2, b2, Wp, bp):
    from concourse.bass_utils import run_bass_kernel_spmd

    nc = _get_nc()
    in_maps = make_in_maps(x, W1, b1, W2, b2, Wp, bp)
    res = run_bass_kernel_spmd(nc, in_maps, core_ids=list(range(NCORES)))
    return postprocess(res.results, bp)
